# revision 15
# baseline (speedup 1.0000x reference)
"""Trainium2 Bass kernel for the self-attention block (nn_Attention).

Reference computation (per batch b, row h):
    f = x @ wf + bf; g = x @ wg + bg; h = x @ wh + bh      (1x1 convs)
    s = g @ f^T (over W); beta = softmax(s, -1); o = beta @ h
    out = gamma * o + x

Sharding: data-parallel over batch B=8, one batch element per NeuronCore.
Per core, each of the 128 rows is an independent [W=128, C=512] block.

v4: fp8e4 DoubleRow matmuls for the projections + a software-pipelined
pair loop tuned against measured engine rates (ACT ~263+1.07/col drain,
DVE ~190+1.18/col, stt ~750, all per the cayman SBUF-access errata).

  - x ships twice: xt8 (fp8, transposed + DoubleRow-interleaved, 8.4MB)
    feeds the PE; x4 (bf16 natural, 16.8MB) is the residual. out bf16.
  - Weights pre-scaled by 64 on the host (fp8 subnormal avoidance); the
    1/64**2 is folded into the exp scale, the 64/|gamma| into the ones
    vector of the Z-matmuls, sign(gamma) into wh, gamma*bh into x4.
  - Rows processed in PAIRS. Critical chain per pair is
    s-MM -> exp -> Z-MM -> recip -> stt; the exp is FIRST in the ACT
    queue and the h work of the NEXT pair (4 DR matmuls + its split
    ACT/DVE drain) is issued in the current step, so the PE and both
    drain engines stay busy while the chain runs.
  - h PSUM is one [128,2,C] tile (2 banks, bufs=1); it is drained in
    the step BEFORE its o-matmuls consume it, which is what lets a
    single buffer rotate without stalling the PE.
  - Z via two N=1 matmuls reusing the at2 halves as stationary (no
    second exp, no accum_out); one paired reciprocal.
"""

import numpy as np
import ml_dtypes

import concourse.bacc as bacc
import concourse.bass as bass
import concourse.mybir as mybir
import concourse.tile as tile

B, H, W, C = 8, 128, 128, 512
CK = C // 8  # 64
N_CORES = 8

F32 = mybir.dt.float32
BF16 = mybir.dt.bfloat16
FP8 = mybir.dt.float8e4
BFDT = ml_dtypes.bfloat16
E4DT = ml_dtypes.float8_e4m3
AF = mybir.ActivationFunctionType
ALU = mybir.AluOpType
DR = mybir.MatmulPerfMode.DoubleRow

WS = 64.0    # host-side weight scale
H_ACT = 384  # columns (of 512) of each h half-drain handled by ScalarE


def row_batch(rows: int) -> int:
    for rb in (4, 2):
        if rows % rb == 0:
            return rb
    return 1


def build_nc(rows: int = H) -> bass.Bass:
    nc = bacc.Bacc(None)
    RB = row_batch(rows)
    nrb = rows // RB
    npair = RB // 2
    assert npair, "rows must be a multiple of 2"
    xt8_d = nc.dram_tensor("xt8", [nrb, 128, RB * C], FP8, kind="ExternalInput")
    x4_d = nc.dram_tensor("x4", [nrb, 128, RB * C], BF16, kind="ExternalInput")
    wh8_d = nc.dram_tensor("wh8", [128, 2 * 2 * C], FP8, kind="ExternalInput")
    wf8_d = nc.dram_tensor("wf8", [128, 2 * 2 * CK], FP8, kind="ExternalInput")
    wg8_d = nc.dram_tensor("wg8", [128, 2 * 2 * CK], FP8, kind="ExternalInput")
    bf64_d = nc.dram_tensor("bf64", [CK, 1], F32, kind="ExternalInput")
    bg64_d = nc.dram_tensor("bg64", [CK, 1], F32, kind="ExternalInput")
    onesg_d = nc.dram_tensor("onesg", [W, 1], BF16, kind="ExternalInput")
    out_d = nc.dram_tensor("out", [nrb, 128, RB * C], BF16, kind="ExternalOutput")

    with tile.TileContext(nc) as tc:
        with (
            tc.tile_pool(name="const", bufs=1) as cpool,
            tc.tile_pool(name="sb_xt", bufs=3) as sb_xt,
            tc.tile_pool(name="sb_x", bufs=3) as sb_x,
            tc.tile_pool(name="sb_fg", bufs=2) as sb_fg,
            tc.tile_pool(name="sb_h", bufs=3) as sb_h,
            tc.tile_pool(name="sb_at", bufs=3) as sb_at,
            tc.tile_pool(name="sb_out", bufs=2) as sb_out,
            tc.tile_pool(name="sb_small", bufs=6) as sb_small,
            tc.tile_pool(name="ps_f", bufs=1, space="PSUM") as ps_f,
            tc.tile_pool(name="ps_g", bufs=1, space="PSUM") as ps_g,
            tc.tile_pool(name="ps_h", bufs=1, space="PSUM") as ps_h,
            tc.tile_pool(name="ps_s", bufs=2, space="PSUM") as ps_s,
            tc.tile_pool(name="ps_o", bufs=2, space="PSUM") as ps_o,
        ):
            wh8_sb = cpool.tile([128, 2, 2, C], FP8)
            nc.sync.dma_start(wh8_sb[:], wh8_d[:])
            wf8_sb = cpool.tile([128, 2, 2, CK], FP8)
            nc.sync.dma_start(wf8_sb[:], wf8_d[:])
            wg8_sb = cpool.tile([128, 2, 2, CK], FP8)
            nc.sync.dma_start(wg8_sb[:], wg8_d[:])
            bf64_sb = cpool.tile([CK, 1], F32)
            nc.sync.dma_start(bf64_sb[:], bf64_d[:])
            bg64_sb = cpool.tile([CK, 1], F32)
            nc.sync.dma_start(bg64_sb[:], bg64_d[:])
            onesg_sb = cpool.tile([W, 1], BF16)
            nc.sync.dma_start(onesg_sb[:], onesg_d[:])

            def start_rb(rb):
                """DMAs + f/g projections for one 4-row batch."""
                st = {}
                st["xt8"] = sb_xt.tile(
                    [128, 2, 2, RB, 128], FP8, tag="xt8", name="xt8_t"
                )
                nc.gpsimd.dma_start(st["xt8"][:], xt8_d[rb])
                st["x4"] = sb_x.tile([128, RB * C], BF16, tag="x4", name="x4_t")
                nc.scalar.dma_start(st["x4"][:], x4_d[rb])
                st["out4"] = sb_out.tile(
                    [128, RB * C], BF16, tag="out4", name="out4_t"
                )
                st["rb"] = rb
                fA = ps_f.tile([CK, RB * 128], F32, tag="fA", name="fA_t")
                gA = ps_g.tile([CK, RB * 128], F32, tag="gA", name="gA_t")
                for j in range(2):
                    nc.tensor.matmul(
                        fA[:], lhsT=wf8_sb[:, j], rhs=st["xt8"][:, j],
                        start=(j == 0), stop=(j == 1), perf_mode=DR,
                    )
                for j in range(2):
                    nc.tensor.matmul(
                        gA[:], lhsT=wg8_sb[:, j], rhs=st["xt8"][:, j],
                        start=(j == 0), stop=(j == 1), perf_mode=DR,
                    )
                st["ft16"] = sb_fg.tile([CK, RB, 128], BF16, tag="ft16", name="ft_t")
                nc.scalar.activation(
                    st["ft16"][:], fA[:], AF.Identity, bias=bf64_sb[:]
                )
                st["gt16"] = sb_fg.tile([CK, RB, 128], BF16, tag="gt16", name="gt_t")
                nc.scalar.activation(
                    st["gt16"][:], gA[:], AF.Identity, bias=bg64_sb[:]
                )
                return st

            def emit_h(st, p):
                """h matmuls for pair p of batch st, plus the split drain."""
                hp = ps_h.tile([128, 2, C], F32, tag="h", name="h_ps")
                for rr in range(2):
                    for j in range(2):
                        nc.tensor.matmul(
                            hp[:, rr], lhsT=st["xt8"][:, j, :, 2 * p + rr, :],
                            rhs=wh8_sb[:, j],
                            start=(j == 0), stop=(j == 1), perf_mode=DR,
                        )
                h2 = sb_h.tile([128, 2, C], BF16, tag="h2", name="h2_t")
                nc.vector.tensor_copy(h2[:, :, H_ACT:C], hp[:, :, H_ACT:C])
                nc.scalar.activation(
                    h2[:, :, 0:H_ACT], hp[:, :, 0:H_ACT], AF.Identity
                )
                return h2

            pairs = [(rb, p) for rb in range(nrb) for p in range(npair)]

            def stage_b1(e):
                """Z-matmuls + reciprocal for a pair whose exp already ran."""
                s_ps, at2 = e["s_ps"], e["at2"]
                for rr in range(2):
                    nc.tensor.matmul(
                        s_ps[:, 256 + rr : 257 + rr],
                        lhsT=at2[:, rr * 128 : (rr + 1) * 128],
                        rhs=onesg_sb[:],
                        start=True, stop=True,
                    )
                scale2 = sb_small.tile([128, 2], F32, tag="scale2", name="sc_t")
                nc.vector.reciprocal(scale2[:], s_ps[:, 256:258])
                e["scale2"] = scale2

            def stage_b2(e):
                """o matmuls + epilogue."""
                st, p, at2, h2, scale2 = e["st"], e["p"], e["at2"], e["h2"], e["scale2"]
                for rr in range(2):
                    r = 2 * p + rr
                    o_ps = ps_o.tile([128, C], F32, tag="o", name="o_ps")
                    nc.tensor.matmul(
                        o_ps[:], lhsT=at2[:, rr * 128 : (rr + 1) * 128],
                        rhs=h2[:, rr], start=True, stop=True,
                    )
                    nc.vector.scalar_tensor_tensor(
                        st["out4"][:, r * C : (r + 1) * C],
                        o_ps[:],
                        scale2[:, rr : rr + 1],
                        st["x4"][:, r * C : (r + 1) * C],
                        ALU.mult,
                        ALU.add,
                    )
                if p == npair - 1:
                    nc.sync.dma_start(out_d[st["rb"]], st["out4"][:])

            cur = start_rb(0)
            h2_next = emit_h(cur, 0)
            prev = None
            for rb, p in pairs:
                st = cur
                h2_this = h2_next
                s_ps = ps_s.tile([128, 258], F32, tag="s", name="s_ps")
                for rr in range(2):
                    r = 2 * p + rr
                    nc.tensor.matmul(
                        s_ps[:, rr * 128 : (rr + 1) * 128],
                        lhsT=st["ft16"][:, r], rhs=st["gt16"][:, r],
                        start=True, stop=True,
                    )
                at2 = sb_at.tile([128, 256], BF16, tag="at2", name="at2_t")
                nc.scalar.activation(
                    at2[:], s_ps[:, 0:256], AF.Exp, scale=1.0 / (WS * WS)
                )
                # prefetch next pair's h (matmuls + drain); start the NEXT
                # rb (DMAs + f/g matmuls + drains) a full pair early so the
                # ACT queue spike spreads over two steps
                if p == 0 and npair == 2 and rb + 1 < nrb:
                    nxt_st = start_rb(rb + 1)
                if p + 1 < npair:
                    h2_next = emit_h(st, p + 1)
                elif rb + 1 < nrb:
                    cur = nxt_st if npair == 2 else start_rb(rb + 1)
                    h2_next = emit_h(cur, 0)
                else:
                    h2_next = None
                if prev is not None:
                    stage_b1(prev)
                    stage_b2(prev)
                prev = {"st": st, "p": p, "s_ps": s_ps, "at2": at2, "h2": h2_this}
            stage_b1(prev)
            stage_b2(prev)
    nc.compile()
    return nc


def make_in_map(x_b: np.ndarray, wf, bf, wg, bg, wh, bh, gamma) -> dict:
    """Host-side input staging for one core (layout/dtype + constant folds)."""
    x_b = np.asarray(x_b, np.float32)
    rows = x_b.shape[0]
    RB = row_batch(rows)
    nrb = rows // RB
    gamma_f = float(np.float32(np.asarray(gamma)))
    sgn = 1.0 if gamma_f >= 0 else -1.0
    ag = max(abs(gamma_f), 1e-30)

    xt8 = np.ascontiguousarray(
        x_b.astype(E4DT)
        .reshape(nrb, RB, W, 4, 128)
        .transpose(0, 4, 3, 1, 2)
        .reshape(nrb, 128, RB * C)
    )
    x_adj = x_b + gamma_f * np.asarray(bh, np.float32)
    x4 = np.ascontiguousarray(
        x_adj.astype(BFDT)
        .reshape(nrb, RB, W, C)
        .transpose(0, 2, 1, 3)
        .reshape(nrb, 128, RB * C)
    )

    def w_dr(w_mat, scale):
        w_mat = np.asarray(w_mat, np.float32) * scale
        m = w_mat.shape[1]
        return np.ascontiguousarray(
            w_mat.astype(E4DT).reshape(4, 128, m).transpose(1, 0, 2).reshape(128, 4 * m)
        )

    return {
        "xt8": xt8,
        "x4": x4,
        "wh8": w_dr(wh, WS * sgn),
        "wf8": w_dr(wf, WS),
        "wg8": w_dr(wg, WS),
        "bf64": np.asarray(bf, np.float32).reshape(CK, 1) * WS,
        "bg64": np.asarray(bg, np.float32).reshape(CK, 1) * WS,
        "onesg": np.full((W, 1), WS / ag, np.float32).astype(BFDT),
    }


def unbatch_out(arr: np.ndarray, rows: int) -> np.ndarray:
    """[nrb, 128, RB*C] device layout -> [rows, W, C] f32."""
    RB = row_batch(rows)
    nrb = rows // RB
    return (
        np.asarray(arr)
        .astype(np.float32)
        .reshape(nrb, 128, RB, C)
        .transpose(0, 2, 1, 3)
        .reshape(rows, W, C)
    )


_NC_CACHE: dict = {}


def run(inputs: dict, trace: bool = False, **run_kwargs):
    """Build (cached), run on 8 cores, return (out, BassKernelResults)."""
    from concourse.bass_utils import run_bass_kernel_spmd

    if "nc" not in _NC_CACHE:
        _NC_CACHE["nc"] = build_nc()
    nc = _NC_CACHE["nc"]
    x = np.asarray(inputs["x"], np.float32)
    in_maps = [
        make_in_map(
            x[b],
            inputs["wf"],
            inputs["bf"],
            inputs["wg"],
            inputs["bg"],
            inputs["wh"],
            inputs["bh"],
            inputs["gamma"],
        )
        for b in range(N_CORES)
    ]
    res = run_bass_kernel_spmd(
        nc, in_maps, list(range(N_CORES)), trace=trace, **run_kwargs
    )
    out = np.stack(
        [unbatch_out(res.results[b]["out"], H) for b in range(N_CORES)], axis=0
    )
    return out, res


def kernel(**inputs) -> np.ndarray:
    out, _ = run(inputs, trace=False)
    return out
